# revision 1
# baseline (speedup 1.0000x reference)
# Contrastive loss (L2-distance scores, margin hinge, mean reduction) on 8
# Trainium2 NeuronCores.
#
# total = mean(cost_s) + mean(cost_im) over the [N, N] score matrix
#   scores[i, j] = -||im_i - s_j||
#   cost_s  = relu(margin + scores - diag_row)   (diag zeroed)
#   cost_im = relu(margin + scores - diag_col)   (diag zeroed)
#
# Identity used on device:  relu(a - d) = max(a, d) - d, so
#   sum(cost_s) + sum(cost_im) = S1 + S2 - 2*SD
#   S1 = sum_ij max(a_i, D_ij), S2 = sum_ij max(b_j, D_ij), SD = sum_ij D_ij
# with a_i = b_i = margin + ||im_i - s_i||.
#
# Sharding: rows of the score matrix across 8 cores ([1024, 8192] slab each).
# s (columns) is replicated but column-ROTATED per core so each core's
# diagonal block lands at local columns [0, 1024) -> one static SPMD program.
# The diagonal is zeroed exactly by subtracting BIG from the PSUM diagonal
# before the sqrt: then max(a, D_ii)=D_ii and the diag contributes 0 to
# S1 + S2 - 2*SD.
#
# Per (m-tile, group-of-2048-cols):
#   PE  : 2x K=128 bf16 matmuls (dot) + 1x K=2 ones-matmul adding a bf16
#         hi/lo split of -||s_j||^2/2  ->  PSUM q = dot - s_sq/2
#   DVE : (group 0 only) in-place add of -BIG*eye on the diagonal block
#   ACT : D = sqrt(-2*q + im_sq_i)  [PSUM->SBUF bf16], accum_out -> SD
#   DVE : tensor_scalar  max(D, a_i)        accum_out -> S1   (4x mode)
#   DVE : scalar_tensor_tensor max(D, b_j)  accum_out -> S2   (2x mode)
# Final: reduce accum columns, partition-sum via ones-matmul, DMA scalar out.
# Host: sum 8 partials, divide by N^2.

import numpy as np
import ml_dtypes

import concourse.bass as bass
import concourse.tile as tile
from concourse import bacc, mybir
from concourse import bass_utils

N = 8192
D = 256
MARGIN = 0.2
NCORES = 8
SLAB = N // NCORES          # 1024 rows per core
MT = SLAB // 128            # 8 m-tiles per core
GROUP = 2048                # columns per PSUM group (4 banks)
NG = N // GROUP             # 4 groups
CHUNK = 512                 # columns per matmul (1 PSUM bank)
NCHUNK = GROUP // CHUNK     # 4 chunks per group
BIG = 5.0e4

BF16 = ml_dtypes.bfloat16
_F = mybir.dt.float32
_B = mybir.dt.bfloat16


def build_module():
    """Trace + compile the per-core Bass module (one SPMD NEFF for 8 cores)."""
    nc = bacc.Bacc("TRN2", num_devices=NCORES)

    imT = nc.dram_tensor("imT", [2, 128, SLAB], _B, kind="ExternalInput")
    sT = nc.dram_tensor("sT", [2, 128, N], _B, kind="ExternalInput")
    fold = nc.dram_tensor("fold", [2, N], _B, kind="ExternalInput")
    brow = nc.dram_tensor("brow", [N], _B, kind="ExternalInput")
    avec = nc.dram_tensor("avec", [128, MT], _F, kind="ExternalInput")
    imsq = nc.dram_tensor("imsq", [128, MT], _F, kind="ExternalInput")
    eyeb = nc.dram_tensor("eyeb", [128, 128], _F, kind="ExternalInput")
    out = nc.dram_tensor("out", [1, 1], _F, kind="ExternalOutput")

    with tile.TileContext(nc) as tc:
        with (
            tc.tile_pool(name="singles", bufs=1) as singles,
            tc.tile_pool(name="dtiles", bufs=3) as dpool,
            tc.tile_pool(name="trash", bufs=2) as tpool,
            tc.tile_pool(name="psum", bufs=2, space="PSUM") as ppool,
        ):
            # ---- resident inputs -------------------------------------------------
            lhs_sb = singles.tile([128, 2, SLAB], _B)
            for k in range(2):
                nc.sync.dma_start(out=lhs_sb[:, k, :], in_=imT.ap()[k])
            rhs_sb = singles.tile([128, 2, N], _B)
            for k in range(2):
                for g in range(NG):
                    cols = slice(g * GROUP, (g + 1) * GROUP)
                    nc.sync.dma_start(out=rhs_sb[:, k, cols], in_=sT.ap()[k, :, cols])
            fold_sb = singles.tile([2, N], _B)
            nc.sync.dma_start(out=fold_sb[:], in_=fold.ap())
            b_sb = singles.tile([128, N], _B)
            brow_bcast = bass.AP(
                tensor=brow.ap().tensor, offset=0, ap=[[0, 128], [1, N]]
            )
            nc.sync.dma_start(out=b_sb[:], in_=brow_bcast)
            avec_sb = singles.tile([128, MT], _F)
            nc.sync.dma_start(out=avec_sb[:], in_=avec.ap())
            imsq_sb = singles.tile([128, MT], _F)
            nc.sync.dma_start(out=imsq_sb[:], in_=imsq.ap())
            eyeb_sb = singles.tile([128, 128], _F)
            nc.sync.dma_start(out=eyeb_sb[:], in_=eyeb.ap())

            ones2 = singles.tile([2, 128], _B)
            nc.vector.memset(ones2[:], 1.0)
            ones_col = singles.tile([128, 1], _F)
            nc.vector.memset(ones_col[:], 1.0)

            accA = singles.tile([128, MT * NG], _F)
            accB = singles.tile([128, MT * NG], _F)
            accD = singles.tile([128, MT * NG], _F)

            # ---- main loop -------------------------------------------------------
            for m in range(MT):
                lhs0 = lhs_sb[:, 0, m * 128 : (m + 1) * 128]
                lhs1 = lhs_sb[:, 1, m * 128 : (m + 1) * 128]
                a_col = avec_sb[:, m : m + 1]
                q_col = imsq_sb[:, m : m + 1]
                for g in range(NG):
                    ps = ppool.tile([128, GROUP], _F, tag="psum")
                    for c in range(NCHUNK):
                        pslice = ps[:, c * CHUNK : (c + 1) * CHUNK]
                        cols = slice(g * GROUP + c * CHUNK, g * GROUP + (c + 1) * CHUNK)
                        nc.tensor.matmul(
                            pslice, lhsT=lhs0, rhs=rhs_sb[:, 0, cols],
                            start=True, stop=False,
                        )
                        nc.tensor.matmul(
                            pslice, lhsT=lhs1, rhs=rhs_sb[:, 1, cols],
                            start=False, stop=False,
                        )
                        nc.tensor.matmul(
                            pslice, lhsT=ones2[:], rhs=fold_sb[:, cols],
                            start=False, stop=True,
                        )
                    if g == 0:
                        # diagonal block of this m-tile: local cols [128m, 128m+128)
                        dslice = ps[:, m * 128 : (m + 1) * 128]
                        nc.vector.tensor_tensor(
                            out=dslice, in0=dslice, in1=eyeb_sb[:],
                            op=mybir.AluOpType.add,
                        )
                    col = m * NG + g
                    dt = dpool.tile([128, GROUP], _B, tag="dt")
                    nc.scalar.activation(
                        out=dt[:], in_=ps[:],
                        func=mybir.ActivationFunctionType.Sqrt,
                        bias=q_col, scale=-2.0,
                        accum_out=accD[:, col : col + 1],
                    )
                    t1 = tpool.tile([128, GROUP], _B, tag="t1")
                    nc.vector.tensor_scalar(
                        out=t1[:], in0=dt[:],
                        scalar1=a_col, scalar2=0.0,
                        op0=mybir.AluOpType.max, op1=mybir.AluOpType.add,
                        accum_out=accA[:, col : col + 1],
                    )
                    t2 = tpool.tile([128, GROUP], _B, tag="t2")
                    nc.vector.scalar_tensor_tensor(
                        out=t2[:], in0=dt[:], scalar=0.0,
                        in1=b_sb[:, g * GROUP : (g + 1) * GROUP],
                        op0=mybir.AluOpType.add, op1=mybir.AluOpType.max,
                        accum_out=accB[:, col : col + 1],
                    )

            # ---- combine ---------------------------------------------------------
            red = singles.tile([128, 4], _F)
            nc.vector.tensor_reduce(
                out=red[:, 0:1], in_=accA[:], axis=mybir.AxisListType.X,
                op=mybir.AluOpType.add,
            )
            nc.vector.tensor_reduce(
                out=red[:, 1:2], in_=accB[:], axis=mybir.AxisListType.X,
                op=mybir.AluOpType.add,
            )
            nc.vector.tensor_reduce(
                out=red[:, 2:3], in_=accD[:], axis=mybir.AxisListType.X,
                op=mybir.AluOpType.add,
            )
            total_col = singles.tile([128, 1], _F)
            nc.vector.tensor_add(total_col[:], red[:, 0:1], red[:, 1:2])
            nc.vector.tensor_scalar(
                out=red[:, 3:4], in0=red[:, 2:3], scalar1=-2.0, scalar2=None,
                op0=mybir.AluOpType.mult,
            )
            nc.vector.tensor_add(total_col[:], total_col[:], red[:, 3:4])

            fps = ppool.tile([1, 1], _F, tag="psum")
            nc.tensor.matmul(fps[:], lhsT=total_col[:], rhs=ones_col[:],
                             start=True, stop=True)
            out_sb = singles.tile([1, 1], _F)
            nc.vector.tensor_copy(out=out_sb[:], in_=fps[:])
            nc.sync.dma_start(out=out.ap(), in_=out_sb[:])

    nc.compile()
    return nc


def prepare_inputs(im: np.ndarray, s: np.ndarray):
    """Host-side sharding + dtype conversion. Returns in_maps for 8 cores."""
    im = np.ascontiguousarray(im, dtype=np.float32)
    s = np.ascontiguousarray(s, dtype=np.float32)

    im64 = im.astype(np.float64)
    s64 = s.astype(np.float64)
    diag_true = np.sqrt(((im64 - s64) ** 2).sum(1))          # [N] exact
    b_full = (MARGIN + diag_true).astype(BF16)               # [N] bf16

    im_q = im.astype(BF16)
    s_q = s.astype(BF16)
    im_sq = (im_q.astype(np.float64) ** 2).sum(1).astype(np.float32)  # [N]
    s_sq = (s_q.astype(np.float64) ** 2).sum(1)                       # [N] f64
    foldv = -0.5 * s_sq
    fold_hi = foldv.astype(np.float32).astype(BF16)
    fold_lo = (foldv - fold_hi.astype(np.float64)).astype(np.float32).astype(BF16)

    eyeb = (np.eye(128, dtype=np.float32) * np.float32(-BIG))

    in_maps = []
    for c in range(NCORES):
        rows = slice(c * SLAB, (c + 1) * SLAB)
        rot = np.roll(np.arange(N), -c * SLAB)
        imT = np.ascontiguousarray(im_q[rows].T.reshape(2, 128, SLAB))
        sT = np.ascontiguousarray(s_q[rot].T.reshape(2, 128, N))
        foldc = np.ascontiguousarray(np.stack([fold_hi[rot], fold_lo[rot]]))
        browc = np.ascontiguousarray(b_full[rot])
        avecc = np.ascontiguousarray(
            b_full[rows].astype(np.float32).reshape(MT, 128).T
        )
        imsqc = np.ascontiguousarray(im_sq[rows].reshape(MT, 128).T)
        in_maps.append(
            {
                "imT": imT,
                "sT": sT,
                "fold": foldc,
                "brow": browc,
                "avec": avecc,
                "imsq": imsqc,
                "eyeb": eyeb,
            }
        )
    return in_maps


_NC_CACHE = None


def get_module():
    global _NC_CACHE
    if _NC_CACHE is None:
        _NC_CACHE = build_module()
    return _NC_CACHE


def kernel(im: np.ndarray, s: np.ndarray) -> np.ndarray:
    nc = get_module()
    in_maps = prepare_inputs(im, s)
    res = bass_utils.run_bass_kernel_spmd(
        nc, in_maps, core_ids=list(range(NCORES))
    )
    total = 0.0
    for c in range(NCORES):
        total += float(res.results[c]["out"][0, 0])
    return np.array(np.float64(total) / (N * N), dtype=np.float32)
